# revision 13
# baseline (speedup 1.0000x reference)
"""ChemGeomFeatEncoder TRN2 kernel.

Strategy: shard edges by OWNER VERTEX across 8 cores (host argsort of
nbr_vids).  Each core owns a contiguous V/8 vertex range and processes the
(sorted, padded) edges pointing into it: gated chem MLP -> segment-sum via
one-hot matmul into PSUM -> per-vertex geom/feat MLPs.  No collectives.

All BatchNorms are folded into the linear weights on the host.  sigmoid is
computed as 0.5+0.5*tanh(x/2) (tanh lives in the silu ACT table set) and
softplus as Ln(exp(x)+1) (exp+ln share a set), so the ACT engine alternates
between exactly two table sets, batched K supertiles at a time.
"""
import numpy as np
import ml_dtypes

import concourse.bacc as bacc
import concourse.mybir as mybir
import concourse.tile as tile
from concourse.bass_utils import run_bass_kernel_spmd

dt = mybir.dt
AF = mybir.ActivationFunctionType
OP = mybir.AluOpType

EPS = 1e-5
NCORES = 8
P = 128          # partitions / tile edge dim
ST = 512         # supertile edge count (4 tiles)
KBATCH = 24      # supertiles per ACT-table phase batch
BF16 = ml_dtypes.bfloat16
DEBUG = False
ACT_CHAIN = True

_cache = {}


def _fold(w, b, bn):
    """y = bn(x@w + b) -> x@w' + b' with eval-mode BN folded in."""
    g, be, m, v = bn[0], bn[1], bn[2], bn[3]
    a = g / np.sqrt(v + EPS)
    return (w * a[None, :]).astype(np.float32), ((b - m) * a + be).astype(np.float32)


def _host_prep(chem_feats, geom_feats, nbr_vids, weights):
    """Sort edges by vertex, build per-core padded streams + folded weights."""
    (w1, b1, bn1, w2, b2, bn2, wg1, bg1, bng1, wg2, bg2, bng2,
     wf1, bf1, bnf1, wf2, bf2, bnf2) = weights
    E, CHEM_IN = chem_feats.shape
    V, GEOM_IN = geom_feats.shape
    H = w1.shape[1]
    VC = V // NCORES
    NSEG = VC // P

    w1f, b1f = _fold(w1, b1, bn1)
    w2f, b2f = _fold(w2, b2, bn2)
    wg1f, bg1f = _fold(wg1, bg1, bng1)
    wg2f, bg2f = _fold(wg2, bg2, bng2)
    wf1f, bf1f = _fold(wf1, bf1, bnf1)
    wf2f, bf2f = _fold(wf2, bf2, bnf2)
    # gate = sigmoid(f)*softplus(c) = 0.5*(sp + tanh(f/2)*sp); fold the 0.5
    # into the h_chem rows of wf1.
    wf1f = wf1f.copy()
    wf1f[:H, :] *= 0.5

    order = np.argsort(nbr_vids, kind="stable")
    svids = nbr_vids[order].astype(np.int64)

    # per-(core,segment) edge counts; common tiles-per-segment across cores
    seg_bounds = np.searchsorted(svids, np.arange(NCORES * NSEG + 1) * P)
    seg_counts = np.diff(seg_bounds).reshape(NCORES, NSEG)
    T_s = np.maximum((seg_counts + P - 1) // P, 1).max(axis=0)  # [NSEG]
    n_tiles = int(T_s.sum())
    pad4 = (-n_tiles) % 4
    T_s = T_s.copy()
    T_s[-1] += pad4
    n_tiles += pad4
    E_pad = n_tiles * P
    n_st = n_tiles // 4

    tile_off = np.zeros(NSEG + 1, dtype=np.int64)
    np.cumsum(T_s, out=tile_off[1:])

    # destination slot for every sorted edge
    # for core c, segment s: edges seg_bounds[c*NSEG+s] .. +cnt  ->
    #   columns tile_off[s]*P .. +cnt of core c's stream
    chemT_pad = np.zeros((NCORES, CHEM_IN, E_pad), dtype=np.float32)
    vrel_pad = np.full((NCORES, n_tiles, P), -1.0, dtype=np.float32)
    chem_sorted = np.ascontiguousarray(chem_feats[order].T)  # [CHEM_IN, E] sorted
    for c in range(NCORES):
        cnts = seg_counts[c]
        starts = seg_bounds[c * NSEG:(c + 1) * NSEG]
        # source index per slot
        dst_col = np.concatenate(
            [tile_off[s] * P + np.arange(cnts[s]) for s in range(NSEG)])
        src_idx = np.concatenate(
            [starts[s] + np.arange(cnts[s]) for s in range(NSEG)])
        chemT_pad[c][:, dst_col] = chem_sorted[:, src_idx]
        vr = np.concatenate(
            [svids[starts[s]:starts[s] + cnts[s]] - (c * VC + s * P)
             for s in range(NSEG)]).astype(np.float32)
        vflat = vrel_pad[c].reshape(-1)
        vflat[dst_col] = vr
    # reshape vrel to [n_st, P, 4] (per-supertile [128, 4] slices)
    vrel_st = np.ascontiguousarray(
        vrel_pad.reshape(NCORES, n_st, 4, P).transpose(0, 1, 3, 2))

    geomT = np.ascontiguousarray(
        geom_feats.reshape(NCORES, VC, GEOM_IN).transpose(0, 2, 1)).astype(np.float32)

    consts = dict(
        w1f=w1f, b1f=b1f.reshape(H, 1),
        w2f_f=w2f[:, :H], w2f_c=w2f[:, H:],
        b2f_f=(0.5 * b2f[:H]).reshape(H, 1),   # tanh((x+b)/2) = tanh(.5x+.5b)
        b2f_c=b2f[H:].reshape(H, 1),
        wg1f=wg1f, bg1f=bg1f.reshape(-1, 1),
        wg2f=wg2f, bg2f=bg2f.reshape(-1, 1),
        wf1f_a=wf1f[:H, :], wf1f_b=wf1f[H:, :], bf1f=bf1f.reshape(H, 1),
        wf2f=wf2f, bf2f=bf2f.reshape(H, 1),
        iota=np.broadcast_to(np.arange(P, dtype=np.float32)[None, :],
                             (P, P)).astype(BF16).copy(),
        ident_bf=np.eye(P, dtype=np.float32).astype(BF16),
        ident_f32=np.eye(P, dtype=np.float32),
    )
    dims = dict(E=E, V=V, H=H, CHEM_IN=CHEM_IN, GEOM_IN=GEOM_IN,
                VC=VC, NSEG=NSEG, n_tiles=n_tiles, n_st=n_st, E_pad=E_pad)
    per_core = dict(chemT=chemT_pad, vrel=vrel_st, geomT=geomT)
    return dims, tuple(int(t) for t in T_s), consts, per_core


def _build_nc(dims, T_s, trace_sim=False):
    H = dims["H"]
    CHEM_IN = dims["CHEM_IN"]
    GEOM_IN = dims["GEOM_IN"]
    VC = dims["VC"]
    NSEG = dims["NSEG"]
    n_tiles = dims["n_tiles"]
    n_st = dims["n_st"]
    E_pad = dims["E_pad"]
    GH = H // 2  # geom hidden = 64

    # tile index -> (segment, first?, last?)
    tile_seg = []
    for s in range(NSEG):
        for k in range(T_s[s]):
            tile_seg.append((s, k == 0, k == T_s[s] - 1))
    assert len(tile_seg) == n_tiles

    nc = bacc.Bacc("TRN2", target_bir_lowering=False)
    tc = tile.TileContext(nc, trace_sim=trace_sim)

    d_chemT = nc.dram_tensor("chemT", [CHEM_IN, E_pad], dt.float32r, kind="ExternalInput")
    d_vrel = nc.dram_tensor("vrel", [n_st, P, 4], dt.float32, kind="ExternalInput")
    d_geomT = nc.dram_tensor("geomT", [GEOM_IN, VC], dt.float32r, kind="ExternalInput")
    d_w1f = nc.dram_tensor("w1f", [CHEM_IN, H], dt.float32r, kind="ExternalInput")
    d_b1f = nc.dram_tensor("b1f", [H, 1], dt.float32, kind="ExternalInput")
    d_w2f_f = nc.dram_tensor("w2f_f", [H, H], dt.float32r, kind="ExternalInput")
    d_w2f_c = nc.dram_tensor("w2f_c", [H, H], dt.float32r, kind="ExternalInput")
    d_b2f_f = nc.dram_tensor("b2f_f", [H, 1], dt.float32, kind="ExternalInput")
    d_b2f_c = nc.dram_tensor("b2f_c", [H, 1], dt.float32, kind="ExternalInput")
    d_wg1f = nc.dram_tensor("wg1f", [GEOM_IN, GH], dt.float32r, kind="ExternalInput")
    d_bg1f = nc.dram_tensor("bg1f", [GH, 1], dt.float32, kind="ExternalInput")
    d_wg2f = nc.dram_tensor("wg2f", [GH, GH], dt.float32r, kind="ExternalInput")
    d_bg2f = nc.dram_tensor("bg2f", [GH, 1], dt.float32, kind="ExternalInput")
    d_wf1f_a = nc.dram_tensor("wf1f_a", [H, H], dt.float32r, kind="ExternalInput")
    d_wf1f_b = nc.dram_tensor("wf1f_b", [GH, H], dt.float32r, kind="ExternalInput")
    d_bf1f = nc.dram_tensor("bf1f", [H, 1], dt.float32, kind="ExternalInput")
    d_wf2f = nc.dram_tensor("wf2f", [H, H], dt.float32r, kind="ExternalInput")
    d_bf2f = nc.dram_tensor("bf2f", [H, 1], dt.float32, kind="ExternalInput")
    d_iota = nc.dram_tensor("iota", [P, P], dt.bfloat16, kind="ExternalInput")
    d_ident_bf = nc.dram_tensor("ident_bf", [P, P], dt.bfloat16, kind="ExternalInput")
    d_ident_f32 = nc.dram_tensor("ident_f32", [P, P], dt.float32, kind="ExternalInput")
    d_out = nc.dram_tensor("out", [VC, H], dt.float32, kind="ExternalOutput")
    d_hcv = (nc.dram_tensor("hcv", [H, VC], dt.float32r, kind="ExternalOutput")
             if DEBUG else None)
    d_hg = (nc.dram_tensor("hg", [GH, VC], dt.float32r, kind="ExternalOutput")
            if DEBUG else None)
    d_x1 = (nc.dram_tensor("x1d", [H, VC], dt.float32r, kind="ExternalOutput")
            if DEBUG else None)

    act_chain = []  # enforce ACT instruction order (table-set batching)

    def act(out_ap, in_ap, func, **kw):
        inst = nc.scalar.activation(out_ap, in_ap, func, **kw)
        act_chain.append(inst)
        return inst

    with tc:
        with (
            tc.tile_pool(name="const", bufs=1) as cpool,
            tc.tile_pool(name="persist", bufs=1) as ppool,
        ):
            # constants
            t_w1f = cpool.tile([CHEM_IN, H], dt.float32r)
            nc.sync.dma_start(out=t_w1f[:], in_=d_w1f[:])
            t_b1f = cpool.tile([H, 1], dt.float32)
            nc.sync.dma_start(out=t_b1f[:], in_=d_b1f[:])
            t_w2f_f = cpool.tile([H, H], dt.float32r)
            nc.sync.dma_start(out=t_w2f_f[:], in_=d_w2f_f[:])
            t_w2f_c = cpool.tile([H, H], dt.float32r)
            nc.sync.dma_start(out=t_w2f_c[:], in_=d_w2f_c[:])
            t_b2f_f = cpool.tile([H, 1], dt.float32)
            nc.sync.dma_start(out=t_b2f_f[:], in_=d_b2f_f[:])
            t_b2f_c = cpool.tile([H, 1], dt.float32)
            nc.sync.dma_start(out=t_b2f_c[:], in_=d_b2f_c[:])
            t_iota = cpool.tile([P, P], dt.bfloat16)
            nc.sync.dma_start(out=t_iota[:], in_=d_iota[:])
            t_ident_bf = cpool.tile([P, P], dt.bfloat16)
            nc.sync.dma_start(out=t_ident_bf[:], in_=d_ident_bf[:])

            # persistent accumul. target: h_chem^T per vertex [H, VC] fp32
            t_hcv = ppool.tile([H, VC], dt.float32r)

            with (
                tc.tile_pool(name="chem_in", bufs=3) as chpool,
                tc.tile_pool(name="h1", bufs=KBATCH + 8) as h1pool,
                tc.tile_pool(name="tnh", bufs=KBATCH + 8) as tpool,
                tc.tile_pool(name="vrel", bufs=3) as vrpool,
                tc.tile_pool(name="etmp", bufs=3) as etpool,
                tc.tile_pool(name="hh", bufs=4) as hhpool,
                tc.tile_pool(name="psA", bufs=2, space="PSUM") as psA,
                tc.tile_pool(name="psB", bufs=2, space="PSUM") as psB,
                tc.tile_pool(name="psT", bufs=2, space="PSUM") as psT,
                tc.tile_pool(name="psS", bufs=2, space="PSUM") as psS,
            ):
                n_batches = (n_st + KBATCH - 1) // KBATCH
                h1_tiles = [None] * n_st
                t_tiles = [None] * n_st
                seg_acc = {}
                for b in range(n_batches):
                    sts = range(b * KBATCH, min((b + 1) * KBATCH, n_st))
                    # ---- phase A: silu+tanh table set ----
                    for st in sts:
                        ct = chpool.tile([CHEM_IN, ST], dt.float32r, tag="ct")
                        nc.sync.dma_start(
                            out=ct[:], in_=d_chemT[:, st * ST:(st + 1) * ST])
                        p1 = psA.tile([P, ST], dt.float32, tag="p1")
                        nc.tensor.matmul(out=p1[:], lhsT=t_w1f[:], rhs=ct[:],
                                         start=True, stop=True)
                        h1 = h1pool.tile([P, ST], dt.float32r, tag="h1")
                        act(h1[:], p1[:], AF.Silu, bias=t_b1f[:, :1])
                        h1_tiles[st] = h1
                        p2f = psB.tile([P, ST], dt.float32, tag="p2")
                        nc.tensor.matmul(out=p2f[:], lhsT=t_w2f_f[:], rhs=h1[:],
                                         start=True, stop=True)
                        tnh = tpool.tile([P, ST], dt.bfloat16, tag="tnh")
                        act(tnh[:], p2f[:], AF.Tanh, bias=t_b2f_f[:, :1], scale=0.5)
                        t_tiles[st] = tnh
                    # ---- phase B: exp+ln table set ----
                    for st in sts:
                        h1 = h1_tiles[st]
                        tnh = t_tiles[st]
                        p2c = psB.tile([P, ST], dt.float32, tag="p2")
                        nc.tensor.matmul(out=p2c[:], lhsT=t_w2f_c[:], rhs=h1[:],
                                         start=True, stop=True)
                        ex = etpool.tile([P, ST], dt.float32, tag="ex")
                        act(ex[:], p2c[:], AF.Exp, bias=t_b2f_c[:, :1])
                        sp = etpool.tile([P, ST], dt.bfloat16, tag="sp")
                        act(sp[:], ex[:], AF.Ln, bias=1.0)
                        g1 = etpool.tile([P, ST], dt.bfloat16, tag="g1")
                        nc.vector.tensor_tensor(out=g1[:], in0=tnh[:], in1=sp[:],
                                                op=OP.mult)
                        g2 = etpool.tile([P, ST], dt.bfloat16, tag="g2")
                        nc.vector.tensor_tensor(out=g2[:], in0=sp[:], in1=g1[:],
                                                op=OP.add)
                        vr = vrpool.tile([P, 4], dt.float32, tag="vr")
                        nc.sync.dma_start(out=vr[:], in_=d_vrel[st])
                        for k in range(4):
                            t_idx = st * 4 + k
                            seg, first, last = tile_seg[t_idx]
                            tr = psT.tile([P, P], dt.bfloat16, tag="tr")
                            nc.tensor.transpose(
                                out=tr[:], in_=g2[:, k * P:(k + 1) * P],
                                identity=t_ident_bf[:])
                            hh = hhpool.tile([P, P], dt.bfloat16, tag="hh")
                            nc.vector.tensor_copy(out=hh[:], in_=tr[:])
                            mm = hhpool.tile([P, P], dt.bfloat16, tag="mm")
                            nc.vector.tensor_scalar(
                                out=mm[:], in0=t_iota[:],
                                scalar1=vr[:, k:k + 1], scalar2=None,
                                op0=OP.is_equal)
                            if first:
                                seg_acc[seg] = psS.tile([P, P], dt.float32,
                                                        tag="segacc", name=f"segacc_{seg}")
                            nc.tensor.matmul(out=seg_acc[seg][:], lhsT=hh[:],
                                             rhs=mm[:], start=first, stop=last)
                            if last:
                                nc.vector.tensor_copy(
                                    out=t_hcv[:, seg * P:(seg + 1) * P],
                                    in_=seg_acc[seg][:])
                                del seg_acc[seg]

            if DEBUG:
                with tc.tile_pool(name="dbg", bufs=2) as dbgp:
                    for s in range(NSEG):
                        dv = dbgp.tile([P, P], dt.float32r, tag="dv")
                        nc.vector.tensor_copy(out=dv[:], in_=t_hcv[:, s * P:(s + 1) * P])
                        nc.sync.dma_start(out=d_hcv[:, s * P:(s + 1) * P], in_=dv[:])

            # ---------------- vertex phase ----------------
            with (
                tc.tile_pool(name="geom_in", bufs=2) as gpool,
                tc.tile_pool(name="vtmp", bufs=3) as vtpool,
                tc.tile_pool(name="vout", bufs=3) as vopool,
                tc.tile_pool(name="psV", bufs=1, space="PSUM") as psV,
                tc.tile_pool(name="vconst", bufs=1) as vcpool,
            ):
                t_wg1f = vcpool.tile([GEOM_IN, GH], dt.float32r)
                nc.sync.dma_start(out=t_wg1f[:], in_=d_wg1f[:])
                t_bg1f = vcpool.tile([GH, 1], dt.float32)
                nc.sync.dma_start(out=t_bg1f[:], in_=d_bg1f[:])
                t_wg2f = vcpool.tile([GH, GH], dt.float32r)
                nc.sync.dma_start(out=t_wg2f[:], in_=d_wg2f[:])
                t_bg2f = vcpool.tile([GH, 1], dt.float32)
                nc.sync.dma_start(out=t_bg2f[:], in_=d_bg2f[:])
                t_wf1f_a = vcpool.tile([H, H], dt.float32r)
                nc.sync.dma_start(out=t_wf1f_a[:], in_=d_wf1f_a[:])
                t_wf1f_b = vcpool.tile([GH, H], dt.float32r)
                nc.sync.dma_start(out=t_wf1f_b[:], in_=d_wf1f_b[:])
                t_bf1f = vcpool.tile([H, 1], dt.float32)
                nc.sync.dma_start(out=t_bf1f[:], in_=d_bf1f[:])
                t_wf2f = vcpool.tile([H, H], dt.float32r)
                nc.sync.dma_start(out=t_wf2f[:], in_=d_wf2f[:])
                t_bf2f = vcpool.tile([H, 1], dt.float32)
                nc.sync.dma_start(out=t_bf2f[:], in_=d_bf2f[:])
                t_ident_f32 = vcpool.tile([P, P], dt.float32)
                nc.sync.dma_start(out=t_ident_f32[:], in_=d_ident_f32[:])

                for base in range(0, VC, ST):
                    W = min(ST, VC - base)
                    sl = slice(base, base + W)
                    gt = gpool.tile([GEOM_IN, W], dt.float32r, tag="gt")
                    nc.sync.dma_start(out=gt[:], in_=d_geomT[:, sl])
                    pg1 = psV.tile([GH, W], dt.float32, tag="pg1")
                    nc.tensor.matmul(out=pg1[:], lhsT=t_wg1f[:], rhs=gt[:],
                                     start=True, stop=True)
                    g1s = vtpool.tile([GH, W], dt.float32r, tag="g1s")
                    act(g1s[:], pg1[:], AF.Silu, bias=t_bg1f[:, :1])
                    pg2 = psV.tile([GH, W], dt.float32, tag="pg2")
                    nc.tensor.matmul(out=pg2[:], lhsT=t_wg2f[:], rhs=g1s[:],
                                     start=True, stop=True)
                    hg = vtpool.tile([GH, W], dt.float32r, tag="hg")
                    act(hg[:], pg2[:], AF.Identity, bias=t_bg2f[:, :1])
                    if DEBUG:
                        nc.sync.dma_start(out=d_hg[:, sl], in_=hg[:])
                    # feat mlp
                    pf1 = psV.tile([H, W], dt.float32, tag="pf1", bufs=2)
                    nc.tensor.matmul(out=pf1[:], lhsT=t_wf1f_a[:],
                                     rhs=t_hcv[:, sl],
                                     start=True, stop=False)
                    nc.tensor.matmul(out=pf1[:], lhsT=t_wf1f_b[:], rhs=hg[:],
                                     start=False, stop=True)
                    x1 = vtpool.tile([H, W], dt.float32r, tag="x1")
                    act(x1[:], pf1[:], AF.Silu, bias=t_bf1f[:, :1])
                    if DEBUG:
                        nc.sync.dma_start(out=d_x1[:, sl], in_=x1[:])
                    pf2 = psV.tile([H, W], dt.float32, tag="pf2", bufs=2)
                    nc.tensor.matmul(out=pf2[:], lhsT=t_wf2f[:], rhs=x1[:],
                                     start=True, stop=True)
                    outT = vtpool.tile([H, W], dt.float32, tag="outT")
                    act(outT[:], pf2[:], AF.Identity, bias=t_bf2f[:, :1])
                    for k in range(W // P):
                        trv = psV.tile([P, P], dt.float32, tag="trv", bufs=2)
                        nc.tensor.transpose(
                            out=trv[:], in_=outT[:, k * P:(k + 1) * P],
                            identity=t_ident_f32[:])
                        ov = vopool.tile([P, H], dt.float32, tag="ov")
                        nc.vector.tensor_copy(out=ov[:], in_=trv[:])
                        nc.sync.dma_start(
                            out=d_out[base + k * P: base + (k + 1) * P, :],
                            in_=ov[:])

    # pin ACT execution order so table-set switches follow the batch plan
    if ACT_CHAIN:
        for a, b2 in zip(act_chain, act_chain[1:]):
            tile.add_dep_helper(a.ins, b2.ins, False, "act table order")

    nc.compile()
    if trace_sim:
        ents = [e for e in tc._perfetto_entries if e[2] is not None]
        if ents:
            t0 = min(e[1] for e in ents)
            t1 = max(e[2] for e in ents)
            print(f"[sim] estimated makespan: {(t1 - t0) / 1000:.1f} us")
            # per-engine busy
            busy = {}
            for e in ents:
                busy[e[0].split('.')[0] if '.' in e[0] else e[6] or '?'] = 0
            nc._sim_makespan_ns = t1 - t0
    return nc


def kernel(chem_feats, geom_feats, nbr_vids,
           w1, b1, bn1, w2, b2, bn2,
           wg1, bg1, bng1, wg2, bg2, bng2,
           wf1, bf1, bnf1, wf2, bf2, bnf2):
    chem_feats = np.asarray(chem_feats, dtype=np.float32)
    geom_feats = np.asarray(geom_feats, dtype=np.float32)
    nbr_vids = np.asarray(nbr_vids)
    weights = tuple(np.asarray(w, dtype=np.float32) for w in (
        w1, b1, bn1, w2, b2, bn2, wg1, bg1, bng1, wg2, bg2, bng2,
        wf1, bf1, bnf1, wf2, bf2, bnf2))

    dims, T_s, consts, per_core = _host_prep(
        chem_feats, geom_feats, nbr_vids, weights)

    key = (dims["E_pad"], T_s)
    if key not in _cache:
        _cache[key] = _build_nc(dims, T_s)
    nc = _cache[key]

    base = {
        "w1f": consts["w1f"], "b1f": consts["b1f"],
        "w2f_f": consts["w2f_f"], "w2f_c": consts["w2f_c"],
        "b2f_f": consts["b2f_f"], "b2f_c": consts["b2f_c"],
        "wg1f": consts["wg1f"], "bg1f": consts["bg1f"],
        "wg2f": consts["wg2f"], "bg2f": consts["bg2f"],
        "wf1f_a": consts["wf1f_a"], "wf1f_b": consts["wf1f_b"],
        "bf1f": consts["bf1f"], "wf2f": consts["wf2f"], "bf2f": consts["bf2f"],
        "iota": consts["iota"], "ident_bf": consts["ident_bf"],
        "ident_f32": consts["ident_f32"],
    }
    in_maps = []
    for c in range(NCORES):
        m = dict(base)
        m["chemT"] = per_core["chemT"][c]
        m["vrel"] = per_core["vrel"][c]
        m["geomT"] = per_core["geomT"][c]
        in_maps.append(m)

    res = run_bass_kernel_spmd(nc, in_maps, core_ids=list(range(NCORES)))
    out = np.concatenate([r["out"] for r in res.results], axis=0)
    return out.astype(np.float32)
